# revision 1
# baseline (speedup 1.0000x reference)
"""Trainium2 Bass kernel for nn_BasisDense: y = einsum('bd,duk,bk->bu', x, kernel, c_prob) + bias.

Strategy:
  - Factorize: t[b,(u,k)] = x @ kernel2d  (kernel2d = kernel.reshape(D, U*K), its
    NATURAL memory layout -> fully contiguous DMA of the kernel tensor), then
    y[b,u] = sum_k t[b,u,k]*c_prob[b,k] + bias[u] (cheap DVE epilogue).
  - Hybrid shard across 8 cores: batch B into 4 x units U into 2 (halves the
    per-core kernel-tensor HBM traffic vs pure batch sharding, keeping the two
    cores of each HBM pair well under the shared 716 GB/s).
  - Matmuls run in float32r (full PE speed; ~1.5e-4 rms rel err vs fp32).
  - Host-side input marshaling: x transposed to [D, BS] (lhsT layout), bias
    broadcast over the 128 partitions. O(B*D + U) work, negligible vs the
    O(B*D*U*K) kernel.
"""
import sys

sys.path.insert(0, "/opt/trn_rl_repo")

import numpy as np
import concourse.bacc as bacc
import concourse.mybir as mybir
import concourse.tile as tile
from concourse import bass_utils

B, D, U, K = 4096, 2048, 2048, 8
NCORES = 8
SHARD_U = 2  # units-dimension shards (1 = pure batch sharding)
SHARD_B = NCORES // SHARD_U
BS = B // SHARD_B  # batch rows per core
USH = U // SHARD_U  # units per core
UKS = USH * K  # fused (u,k) output columns per core
NFREE = 512  # matmul moving free dim (fp32 max, 1 PSUM bank)
NT = UKS // NFREE  # n-tiles
DT = D // 128  # contraction tiles
BT = BS // 128  # batch partition-tiles per core
UPT = NFREE // K  # u-columns produced per n-tile
KT_BUFS = 3

_CACHE = {}


def _build():
    nc = bacc.Bacc("TRN2", target_bir_lowering=False, debug=False, num_devices=NCORES)
    f32 = mybir.dt.float32
    f32r = mybir.dt.float32r

    xt = nc.dram_tensor("xt", [D, BS], f32r, kind="ExternalInput").ap()
    cp = nc.dram_tensor("cp", [BS, K], f32, kind="ExternalInput").ap()
    kern = nc.dram_tensor("kern", [D, USH, K], f32r, kind="ExternalInput").ap()
    biasr = nc.dram_tensor("biasr", [128, USH], f32, kind="ExternalInput").ap()
    y = nc.dram_tensor("y", [BS, USH], f32, kind="ExternalOutput").ap()

    # [128 d-partition, DT, UKS] view of this core's kernel2d shard
    kern2d = kern.rearrange("(t p) u k -> p t (u k)", p=128)

    with tile.TileContext(nc) as tc:
        with (
            tc.tile_pool(name="const", bufs=1) as constp,
            tc.tile_pool(name="kt", bufs=KT_BUFS) as ktp,
            tc.tile_pool(name="mps", bufs=8, space="PSUM") as mps,
            tc.tile_pool(name="ep", bufs=4) as epp,
            tc.tile_pool(name="yp", bufs=16) as ypp,
        ):
            xT = constp.tile([128, DT, BS], f32r)  # [d-part, d-tile, b]
            c_rep = constp.tile([128, BT, NFREE], f32)
            bias_rep = constp.tile([128, USH], f32)

            # xT rides the gpsimd (SWDGE) queue, off the two HWDGE queues
            # that carry the kernel-chunk stream
            xt_v = xt.rearrange("(t p) b -> p t b", p=128)
            c_nat = constp.tile([128, BT, K], f32)
            nc.scalar.dma_start(c_nat, cp.rearrange("(bt p) k -> p bt k", p=128))
            for t in range(DT):
                nc.gpsimd.dma_start(xT[:, t, :], xt_v[:, t, :])
            # replicate c_prob 64x along the free dim on the DVE (tiny)
            for bt in range(BT):
                nc.vector.tensor_copy(c_rep[:, bt, 0:K], c_nat[:, bt, :])
                s = K
                while s < NFREE:
                    nc.vector.tensor_copy(c_rep[:, bt, s : 2 * s], c_rep[:, bt, 0:s])
                    s *= 2

            for n in range(NT):
                kt = ktp.tile([128, DT, NFREE], f32r, tag="kt")
                # per-d-tile chunk DMAs (256KB each): the t-th matmul can
                # start as soon as chunk t lands, alternating across queues
                for t in range(DT):
                    eng = nc.sync if t % 2 == 0 else nc.scalar
                    eng.dma_start(
                        kt[:, t, :],
                        kern2d[:, t, n * NFREE : (n + 1) * NFREE],
                    )
                if n == 0:
                    # queued behind n=0's kernel chunks (frees the scalar
                    # queue's critical window) but emitted before any reader
                    nc.scalar.dma_start(bias_rep, biasr)
                for bt in range(BT):
                    acc = mps.tile([128, NFREE], f32, tag="acc")
                    for t in range(DT):
                        nc.tensor.matmul(
                            acc,
                            xT[:, t, bt * 128 : (bt + 1) * 128],
                            kt[:, t, :],
                            start=(t == 0),
                            stop=(t == DT - 1),
                        )
                    # epilogue: y[b, u] = sum_k acc[b, (u,k)] * c[b, k] + bias[u]
                    tmp = epp.tile([128, NFREE], f32, tag="tmp")
                    nc.vector.tensor_mul(tmp, acc, c_rep[:, bt, :])
                    yt = ypp.tile([128, UPT], f32, tag="yt")
                    nc.vector.tensor_reduce(
                        yt,
                        tmp.rearrange("p (u k) -> p u k", k=K),
                        axis=mybir.AxisListType.X,
                        op=mybir.AluOpType.add,
                    )
                    yf = ypp.tile([128, UPT], f32, tag="yf")
                    nc.vector.tensor_add(yf, yt, bias_rep[:, n * UPT : (n + 1) * UPT])
                    # output DMAs ride the scalar engine's HWDGE queue
                    nc.scalar.dma_start(
                        y[bt * 128 : (bt + 1) * 128, n * UPT : (n + 1) * UPT],
                        yf,
                    )
    nc.compile()
    return nc


def _in_maps(x, c_prob, kernel, bias):
    x = np.ascontiguousarray(x, dtype=np.float32)
    c_prob = np.ascontiguousarray(c_prob, dtype=np.float32)
    kernel = np.ascontiguousarray(kernel, dtype=np.float32)
    bias = np.ascontiguousarray(bias, dtype=np.float32)
    maps = []
    for c in range(NCORES):
        bq, uh = c % SHARD_B, c // SHARD_B
        xs = x[bq * BS : (bq + 1) * BS]
        maps.append(
            {
                "xt": np.ascontiguousarray(xs.T),
                "cp": c_prob[bq * BS : (bq + 1) * BS],
                "kern": np.ascontiguousarray(kernel[:, uh * USH : (uh + 1) * USH, :]),
                "biasr": np.ascontiguousarray(
                    np.broadcast_to(bias[uh * USH : (uh + 1) * USH], (128, USH))
                ),
            }
        )
    return maps


def kernel(x, c_prob, kernel, bias):
    if "nc" not in _CACHE:
        _CACHE["nc"] = _build()
    nc = _CACHE["nc"]
    res = bass_utils.run_bass_kernel_spmd(
        nc, _in_maps(x, c_prob, kernel, bias), list(range(NCORES))
    )
    out = np.empty((B, U), dtype=np.float32)
    for c in range(NCORES):
        bq, uh = c % SHARD_B, c // SHARD_B
        out[bq * BS : (bq + 1) * BS, uh * USH : (uh + 1) * USH] = res.results[c]["y"]
    return out



# revision 5
# speedup vs baseline: 1.0901x; 1.0901x over previous
"""Trainium2 Bass kernel for nn_BasisDense: y = einsum('bd,duk,bk->bu', x, kernel, c_prob) + bias.

Strategy:
  - Factorize: t[b,(u,k)] = x @ kernel2d  (kernel2d = kernel.reshape(D, U*K), its
    NATURAL memory layout -> fully contiguous DMA of the kernel tensor), then
    y[b,u] = sum_k t[b,u,k]*c_prob[b,k] + bias[u] (cheap DVE epilogue).
  - Hybrid shard across 8 cores: batch B into 4 x units U into 2 (halves the
    per-core kernel-tensor HBM traffic vs pure batch sharding, keeping the two
    cores of each HBM pair well under the shared 716 GB/s).
  - Matmuls run in float32r (full PE speed; ~1.5e-4 rms rel err vs fp32).
  - Host-side input marshaling: x transposed to [D, BS] (lhsT layout), bias
    broadcast over the 128 partitions. O(B*D + U) work, negligible vs the
    O(B*D*U*K) kernel.
"""
import sys

sys.path.insert(0, "/opt/trn_rl_repo")

import numpy as np
import concourse.bacc as bacc
import concourse.mybir as mybir
import concourse.tile as tile
from concourse import bass_utils

B, D, U, K = 4096, 2048, 2048, 8
NCORES = 8
SHARD_U = 2  # units-dimension shards (1 = pure batch sharding)
SHARD_B = NCORES // SHARD_U
BS = B // SHARD_B  # batch rows per core
USH = U // SHARD_U  # units per core
UKS = USH * K  # fused (u,k) output columns per core
NFREE = 512  # matmul moving free dim (fp32 max, 1 PSUM bank)
NT = UKS // NFREE  # n-tiles
DT = D // 128  # contraction tiles
BT = BS // 128  # batch partition-tiles per core
UPT = NFREE // K  # u-columns produced per n-tile
KT_BUFS = 3

_CACHE = {}


def _build():
    nc = bacc.Bacc("TRN2", target_bir_lowering=False, debug=False, num_devices=NCORES)
    f32 = mybir.dt.float32
    bf16 = mybir.dt.bfloat16

    xt = nc.dram_tensor("xt", [D, BS], bf16, kind="ExternalInput").ap()
    cp = nc.dram_tensor("cp", [BS, K], f32, kind="ExternalInput").ap()
    kern = nc.dram_tensor("kern", [D, USH, K], bf16, kind="ExternalInput").ap()
    biasr = nc.dram_tensor("biasr", [128, USH], f32, kind="ExternalInput").ap()
    y = nc.dram_tensor("y", [BS, USH], f32, kind="ExternalOutput").ap()

    # [128 d-partition, DT, UKS] view of this core's kernel2d shard
    kern2d = kern.rearrange("(t p) u k -> p t (u k)", p=128)

    with tile.TileContext(nc) as tc:
        with (
            tc.tile_pool(name="const", bufs=1) as constp,
            tc.tile_pool(name="kt", bufs=KT_BUFS) as ktp,
            tc.tile_pool(name="mps", bufs=8, space="PSUM") as mps,
            tc.tile_pool(name="ep", bufs=4) as epp,
            tc.tile_pool(name="yp", bufs=16) as ypp,
        ):
            xT = constp.tile([128, DT, BS], bf16)  # [d-part, d-tile, b]
            c_rep = constp.tile([128, BT, NFREE], f32)
            bias_rep = constp.tile([128, USH], f32)

            # xT rides the gpsimd (SWDGE) queue, off the two HWDGE queues
            # that carry the kernel-chunk stream
            xt_v = xt.rearrange("(t p) b -> p t b", p=128)
            c_nat = constp.tile([128, BT, K], f32)
            nc.scalar.dma_start(c_nat, cp.rearrange("(bt p) k -> p bt k", p=128))
            for t in range(DT):
                nc.gpsimd.dma_start(xT[:, t, :], xt_v[:, t, :])
            # replicate c_prob 64x along the free dim on the DVE (tiny)
            for bt in range(BT):
                nc.vector.tensor_copy(c_rep[:, bt, 0:K], c_nat[:, bt, :])
                s = K
                while s < NFREE:
                    nc.vector.tensor_copy(c_rep[:, bt, s : 2 * s], c_rep[:, bt, 0:s])
                    s *= 2

            for n in range(NT):
                kt = ktp.tile([128, DT, NFREE], bf16, tag="kt")
                # per-d-tile chunk DMAs (256KB each): the t-th matmul can
                # start as soon as chunk t lands, alternating across queues
                for t in range(DT):
                    eng = nc.sync if t % 2 == 0 else nc.scalar
                    eng.dma_start(
                        kt[:, t, :],
                        kern2d[:, t, n * NFREE : (n + 1) * NFREE],
                    )
                if n == 0:
                    # queued behind n=0's kernel chunks (frees the scalar
                    # queue's critical window) but emitted before any reader
                    nc.scalar.dma_start(bias_rep, biasr)
                for bt in range(BT):
                    acc = mps.tile([128, NFREE], f32, tag="acc")
                    for t in range(DT):
                        nc.tensor.matmul(
                            acc,
                            xT[:, t, bt * 128 : (bt + 1) * 128],
                            kt[:, t, :],
                            start=(t == 0),
                            stop=(t == DT - 1),
                        )
                    # epilogue: y[b, u] = sum_k acc[b, (u,k)] * c[b, k] + bias[u]
                    tmp = epp.tile([128, NFREE], f32, tag="tmp")
                    nc.vector.tensor_mul(tmp, acc, c_rep[:, bt, :])
                    yt = ypp.tile([128, UPT], f32, tag="yt")
                    nc.vector.tensor_reduce(
                        yt,
                        tmp.rearrange("p (u k) -> p u k", k=K),
                        axis=mybir.AxisListType.X,
                        op=mybir.AluOpType.add,
                    )
                    yf = ypp.tile([128, UPT], f32, tag="yf")
                    nc.vector.tensor_add(yf, yt, bias_rep[:, n * UPT : (n + 1) * UPT])
                    # output DMAs ride the scalar engine's HWDGE queue
                    nc.scalar.dma_start(
                        y[bt * 128 : (bt + 1) * 128, n * UPT : (n + 1) * UPT],
                        yf,
                    )
    nc.compile()
    return nc


def _in_maps(x, c_prob, kernel, bias):
    import ml_dtypes

    bf16 = ml_dtypes.bfloat16
    x = np.ascontiguousarray(x, dtype=np.float32)
    c_prob = np.ascontiguousarray(c_prob, dtype=np.float32)
    kernel = np.ascontiguousarray(kernel, dtype=np.float32)
    bias = np.ascontiguousarray(bias, dtype=np.float32)
    maps = []
    for c in range(NCORES):
        bq, uh = c % SHARD_B, c // SHARD_B
        xs = x[bq * BS : (bq + 1) * BS]
        maps.append(
            {
                "xt": np.ascontiguousarray(xs.T).astype(bf16),
                "cp": c_prob[bq * BS : (bq + 1) * BS],
                "kern": np.ascontiguousarray(
                    kernel[:, uh * USH : (uh + 1) * USH, :]
                ).astype(bf16),
                "biasr": np.ascontiguousarray(
                    np.broadcast_to(bias[uh * USH : (uh + 1) * USH], (128, USH))
                ),
            }
        )
    return maps


def kernel(x, c_prob, kernel, bias):
    if "nc" not in _CACHE:
        _CACHE["nc"] = _build()
    nc = _CACHE["nc"]
    res = bass_utils.run_bass_kernel_spmd(
        nc, _in_maps(x, c_prob, kernel, bias), list(range(NCORES))
    )
    out = np.empty((B, U), dtype=np.float32)
    for c in range(NCORES):
        bq, uh = c % SHARD_B, c // SHARD_B
        out[bq * BS : (bq + 1) * BS, uh * USH : (uh + 1) * USH] = res.results[c]["y"]
    return out

